# revision 26
# baseline (speedup 1.0000x reference)
"""BinaryNormalizedConv2d on 8 Trainium2 NeuronCores — 1D Winograd F(2,3).

Reference computation (per full input):
  Wq = (w > mean(w)), bq = (b > mean(b))          # {0,1} f32
  z  = conv2d(x, Wq, stride 1, pad 1) + bq
  z  = (z - mean_b(z)) / (sqrt(var_b(z, ddof=1)) + 1e-5)   # per-sample over (C,H,W)
  out = relu(z)

Sharding: data-parallel over batch (32 -> 4 per core), weights replicated.

Device kernel (per core, B=4, Cin=128, Cout=256, H=W=56):
  - Winograd F(2,3) along the width axis: for each output row and kh tap,
    2 output columns take 4 transform-domain multiplies instead of 6.
    Transforms are exact for binary weights (Gw in {0,+-.5,1,1.5}); the
    input transform V = B^T d is precomputed on HOST and shipped (bf16).
  - To keep the PSUM->SBUF inverse transform off the critical path, the
    PE computes THREE banks per (sample, cout-half, 14-row group):
      P = M0+M1 (6 mm), Q = M2 (3 mm), R = M1-M3 (6 mm, -Gw3 preneg)
    15 matmuls of free 392 vs 18-equivalent direct: 1.2x PE saving, but
    the combine is only 2 VectorE ops (z_even = Q+P, z_odd = -Q+R) off
    one ScalarE Copy of Q, so no engine exceeds the PE.
  - Stats per sample like the direct version (row sums via accum_out,
    sumsq via ScalarE Square, partition reduce via ones-matmul), but
    emission of the stats chain is DEFERRED into the next sample's
    group loop so neither the PE nor ScalarE ever head-block on it.
  - normalize+relu chunks (bf16 out, host converts) are spread over the
    next sample's groups, alternating ScalarE / VectorE and sync/gpsimd
    DMA queues; the last sample uses fine chunks to shorten the tail.
"""

import numpy as np
import ml_dtypes
from contextlib import ExitStack

# ---- problem constants (hardcoded per contract) ----
B_FULL, CIN, H, W = 32, 128, 56, 56
COUT, KK = 256, 3
N_CORES = 8
B = B_FULL // N_CORES          # 4 samples per core
HP = H + 2                     # 58 padded rows
NT = W // 2                    # 28 winograd tiles per row
VROW = 4 * NT                  # 112 transformed values per padded row
VS = HP * VROW                 # 6496 per-sample stride in V
RG = 14                        # rows per group
NG = H // RG                   # 4 groups per half
NFREE = RG * NT                # 392 matmul free size
HW = H * W                     # 3136
NELEM = COUT * HW              # 802816 elements per sample for stats
EPS = 1e-5

_CACHE = {}
TRACE = False                  # set by test.py to collect an NTFF profile
TRACE_DIR = None
LAST_RESULTS = None


def _emit(nc, tc, v_d, w_d, bq_d, y_d):
    import concourse.mybir as mybir

    f32 = mybir.dt.float32
    bf16 = mybir.dt.bfloat16
    AF = mybir.ActivationFunctionType
    OP = mybir.AluOpType
    AX = mybir.AxisListType

    with ExitStack() as ctx:
        const_pool = ctx.enter_context(tc.tile_pool(name="const", bufs=1))
        vpool = ctx.enter_context(tc.tile_pool(name="v", bufs=1))
        zpool = ctx.enter_context(tc.tile_pool(name="z", bufs=3))
        cpool = ctx.enter_context(tc.tile_pool(name="c", bufs=3))
        sqpool = ctx.enter_context(tc.tile_pool(name="sq", bufs=2))
        stpool = ctx.enter_context(tc.tile_pool(name="st", bufs=2))
        npool = ctx.enter_context(tc.tile_pool(name="nt", bufs=2))
        outpool = ctx.enter_context(tc.tile_pool(name="out", bufs=5))
        cpsum = ctx.enter_context(tc.tile_pool(name="cps", bufs=7, space="PSUM"))
        spsum = ctx.enter_context(tc.tile_pool(name="sps", bufs=1, space="PSUM"))

        # ---- constants ----
        scr = const_pool.tile([128, NFREE], bf16)
        nc.gpsimd.memset(scr[:], 0.0)        # early memset so warmups start ASAP
        w_sb = const_pool.tile([CIN, 24 * 128], bf16)
        bq_sb = const_pool.tile([128, 3], f32)
        ones = const_pool.tile([128, 128], f32)
        nc.vector.memset(ones[:], 1.0)

        # startup DMA ladder: sync carries ONLY what group 0 needs (h=0
        # weights + V rows 0-15) so it lands as the warmups finish; the
        # gpsimd SWDGE queue leads with the rest of sample 0's V, then
        # streams the remaining samples; scalar takes the h=1 weights.
        v_sb = vpool.tile([CIN, B * VS], bf16)
        nc.gpsimd.dma_start(w_sb[:, 0:12 * 128], w_d[:, 0:12 * 128])
        nc.sync.dma_start(v_sb[:, 8 * VROW:16 * VROW], v_d[:, 8 * VROW:16 * VROW])
        nc.scalar.dma_start(v_sb[:, 0:8 * VROW], v_d[:, 0:8 * VROW])
        nc.scalar.dma_start(w_sb[:, 12 * 128:], w_d[:, 12 * 128:])
        nc.scalar.dma_start(bq_sb[:], bq_d[:])
        nc.gpsimd.dma_start(v_sb[:, 16 * VROW:32 * VROW], v_d[:, 16 * VROW:32 * VROW])
        nc.gpsimd.dma_start(v_sb[:, 32 * VROW:VS], v_d[:, 32 * VROW:VS])
        vcut = 30 * VROW
        for b in range(1, B):
            nc.gpsimd.dma_start(v_sb[:, b * VS:b * VS + vcut],
                                v_d[:, b * VS:b * VS + vcut])
            nc.gpsimd.dma_start(v_sb[:, b * VS + vcut:(b + 1) * VS],
                                v_d[:, b * VS + vcut:(b + 1) * VS])

        # PE warm-up: dummy matmuls ramp the HAM clock gate to 8/8 (2.4GHz)
        # and keep the PE busy until the first inputs land (~15us), so the
        # real conv starts at full clock.
        for _ in range(16):
            dzt = cpsum.tile([128, NFREE], f32, tag="m")
            nc.tensor.matmul(dzt[:], scr[:, 0:128], scr[:], start=True, stop=True)

        # [p, (b y), k, t] view of V for matmul rhs slicing
        v3 = v_sb[:].rearrange("p (y k t) -> p y k t", k=4, t=NT)

        # bank plans: (V k-index, weight block base) per matmul triplet.
        # 15mm: P=M0+M1, Q=M2, R=M1-M3 (combine: 2 DVE ops off Copy(Q))
        # 12mm: M0, M1, M2, -M3 separate (combine: 4 DVE ops off Copy(M1));
        # groups alternate so PSUM pairs fill exactly 7 banks and the PE
        # averages 13.5 matmuls/group while VectorE averages 3 combines.
        PLAN15 = [[(0, 0), (1, 3)], [(2, 6)], [(1, 3), (3, 9)]]
        PLAN12 = [[(0, 0)], [(1, 3)], [(2, 6)], [(3, 9)]]

        state = {}          # per-sample tiles for deferred stats
        pending = []        # normalize chunks awaiting drain slots

        def emit_norm(item):
            eng, h, c0, ln, b_idx, st = item
            scal, b2, z_sb = st["scal"], st["b2"], st["z"]
            zsrc = z_sb[:, h * HW + c0: h * HW + c0 + ln]
            zn = outpool.tile([128, 1848], bf16, tag="zn")
            tail = b_idx == B - 1
            if eng == "act":
                nc.scalar.activation(zn[:, 0:ln], zsrc, AF.Relu,
                                     bias=b2[:, h:h + 1], scale=scal[:, 6:7])
                qs = [nc.sync]
            else:
                tmp = npool.tile([128, 1848], f32, tag="ntmp")
                nc.vector.tensor_scalar(out=tmp[:, 0:ln], in0=zsrc,
                                        scalar1=scal[:, 6:7],
                                        scalar2=b2[:, h:h + 1],
                                        op0=OP.mult, op1=OP.add)
                nc.vector.tensor_scalar_max(zn[:, 0:ln], tmp[:, 0:ln], 0.0)
                qs = [nc.sync] if tail else [nc.gpsimd]
            nsub = len(qs)
            for si, q in enumerate(qs):
                s0, s1 = si * ln // nsub, (si + 1) * ln // nsub
                q.dma_start(
                    y_d[b_idx, h * 128:(h + 1) * 128, c0 + s0:c0 + s1],
                    zn[:, s0:s1])

        def emit_stats_a(st):
            stats = stpool.tile([128, 6], f32, tag="stats")
            st["stats"] = stats
            nc.vector.tensor_reduce(
                stats[:, 0:2],
                st["rsums"][:].rearrange("p (h y) -> p h y", y=8),
                axis=AX.X, op=OP.add)
            qv = st["qsums"][:].rearrange("p (h y) -> p h y", y=4)
            if st["b"] < B - 1:
                qv = qv[:, :, 0:2]
            nc.vector.tensor_reduce(stats[:, 2:4], qv, axis=AX.X, op=OP.add)
            # BR' = 2 * S_h * bq_h
            nc.vector.scalar_tensor_tensor(
                out=stats[:, 4:6], in0=stats[:, 0:2], scalar=2.0,
                in1=bq_sb[:, 0:2], op0=OP.mult, op1=OP.mult)

        def emit_stats_b(st):
            stats = st["stats"]
            st_ps = spsum.tile([128, 6], f32, tag="stps")
            nc.tensor.matmul(st_ps[:], ones[:], stats[:], start=True, stop=True)
            sb_st = stpool.tile([128, 6], f32, tag="sbst")
            nc.vector.tensor_copy(sb_st[:], st_ps[:])
            # scal cols: 1 SStot, 2 tmp, 3 Stot, 4 mean, 5 var, 6 inv, 7 sd/tmp
            scal = stpool.tile([128, 8], f32, tag="scal")
            c1 = bq_sb[:, 2:3]
            nc.vector.scalar_tensor_tensor(
                out=scal[:, 3:4], in0=sb_st[:, 0:1], scalar=c1,
                in1=sb_st[:, 1:2], op0=OP.add, op1=OP.add)      # Stot = S0+S1+C1
            nc.vector.tensor_scalar_mul(scal[:, 4:5], scal[:, 3:4], 1.0 / NELEM)
            nc.vector.scalar_tensor_tensor(
                out=scal[:, 2:3], in0=scal[:, 3:4],
                scalar=1.0 / (float(NELEM) * (NELEM - 1)),
                in1=scal[:, 3:4], op0=OP.mult, op1=OP.mult)     # Stot^2/(N(N-1))
            nc.vector.scalar_tensor_tensor(
                out=scal[:, 1:2], in0=sb_st[:, 2:3], scalar=c1,
                in1=sb_st[:, 3:4], op0=OP.add, op1=OP.add)      # Q0+Q1+C1
            nc.vector.tensor_tensor(scal[:, 0:1], sb_st[:, 4:5], sb_st[:, 5:6],
                                    op=OP.add)                  # 2BR
            nc.vector.tensor_tensor(scal[:, 1:2], scal[:, 1:2], scal[:, 0:1],
                                    op=OP.add)                  # SStot
            nc.vector.scalar_tensor_tensor(
                out=scal[:, 5:6], in0=scal[:, 1:2], scalar=1.0 / (NELEM - 1),
                in1=scal[:, 2:3], op0=OP.mult, op1=OP.subtract)  # var
            nc.scalar.sqrt(scal[:, 7:8], scal[:, 5:6])
            nc.vector.reciprocal(scal[:, 6:7], scal[:, 7:8])     # inv = 1/sd
            b2 = stpool.tile([128, 2], f32, tag="b2")
            for h in range(2):
                nc.vector.scalar_tensor_tensor(
                    out=b2[:, h:h + 1], in0=bq_sb[:, h:h + 1], scalar=scal[:, 4:5],
                    in1=scal[:, 6:7], op0=OP.subtract, op1=OP.mult)
            st["scal"] = scal
            st["b2"] = b2
            b_idx = st["b"]
            if b_idx < B - 1:
                for h, ck, eng in ((1, 1, "dve"), (0, 0, "act"),
                                   (0, 1, "act"), (1, 0, "act")):
                    pending.append((eng, h, ck * 1568, 1568, b_idx, st))
            else:
                # tail: both engines in parallel; h0 drains via gpsimd
                # while h1 goes on the low-latency sync queue
                for h in range(2):
                    pending.append(("act", h, 0, 1848, b_idx, st))
                    pending.append(("dve", h, 1848, 1288, b_idx, st))

        for b in range(B):
            z_sb = zpool.tile([128, 2 * HW], f32, tag="z")
            zv = z_sb[:].rearrange("p (r t two) -> p r t two", t=NT, two=2)
            rsums = stpool.tile([128, 16], f32, tag="rsums")
            qsums = stpool.tile([128, 8], f32, tag="qsums")
            prev = state.get("st")
            cur = {"b": b, "z": z_sb, "rsums": rsums, "qsums": qsums}
            state["st"] = cur

            for gi in range(2 * NG):
                h, g = gi // NG, gi % NG
                # last sample's final groups go combine-light (15mm) so
                # VectorE has no backlog when the conv finishes
                is15 = gi % 2 == 1 or (b == B - 1 and gi == 6)
                plan = PLAN15 if is15 else PLAN12
                banks = []
                for nb, kspec in enumerate(plan):
                    mt = cpsum.tile([128, NFREE], f32, tag="m", name=f"mb{nb}")
                    banks.append(mt)
                    m3 = mt[:].rearrange("p (r t) -> p r t", t=NT)
                    nmm = 3 * len(kspec)
                    j = 0
                    for k, base in kspec:
                        for kh in range(3):
                            y0 = b * HP + RG * g + kh
                            rhs = v3[:, y0:y0 + RG, k:k + 1, :].rearrange(
                                "p r one t -> p r (one t)")
                            wi = (h * 12 + base + kh) * 128
                            nc.tensor.matmul(m3, w_sb[:, wi:wi + 128], rhs,
                                             start=(j == 0), stop=(j == nmm - 1))
                            j += 1

                # ---- inverse transform / evacuation ----
                c = cpool.tile([128, NFREE], f32, tag="c")
                r0 = h * 56 + RG * g
                ze = zv[:, r0:r0 + RG, :, 0:1]
                zo = zv[:, r0:r0 + RG, :, 1:2]
                c3 = c[:].rearrange("p (r t one) -> p r t one", t=NT, one=1)
                col = h * 8 + g * 2
                if is15:
                    pb, qb, rb = banks
                    nc.scalar.activation(c[:], qb[:], AF.Copy)
                    p3 = pb[:].rearrange("p (r t one) -> p r t one", t=NT, one=1)
                    r3 = rb[:].rearrange("p (r t one) -> p r t one", t=NT, one=1)
                    nc.vector.scalar_tensor_tensor(
                        out=ze, in0=c3, scalar=1.0, in1=p3,
                        op0=OP.mult, op1=OP.add, accum_out=rsums[:, col:col + 1])
                    nc.vector.scalar_tensor_tensor(
                        out=zo, in0=c3, scalar=-1.0, in1=r3,
                        op0=OP.mult, op1=OP.add, accum_out=rsums[:, col + 1:col + 2])
                else:
                    m0b, m1b, m2b, m3nb = banks
                    nc.vector.tensor_copy(c[:], m1b[:])
                    u = cpool.tile([128, NFREE], f32, tag="u")
                    t2 = cpool.tile([128, NFREE], f32, tag="u")
                    nc.vector.tensor_tensor(u[:], m0b[:], c[:], op=OP.add)
                    nc.vector.tensor_tensor(t2[:], c[:], m3nb[:], op=OP.add)
                    u3 = u[:].rearrange("p (r t one) -> p r t one", t=NT, one=1)
                    t23 = t2[:].rearrange("p (r t one) -> p r t one", t=NT, one=1)
                    m23 = m2b[:].rearrange("p (r t one) -> p r t one", t=NT, one=1)
                    nc.vector.scalar_tensor_tensor(
                        out=ze, in0=u3, scalar=1.0, in1=m23,
                        op0=OP.mult, op1=OP.add, accum_out=rsums[:, col:col + 1])
                    nc.vector.scalar_tensor_tensor(
                        out=zo, in0=t23, scalar=1.0, in1=m23,
                        op0=OP.mult, op1=OP.subtract,
                        accum_out=rsums[:, col + 1:col + 2])

                if b == B - 1:
                    # fine sq chunks: keeps the last chunk (critical for
                    # the tail stats latency) small; the very last one on
                    # VectorE so it overlaps ScalarE's g6 square
                    sq = sqpool.tile([128, HW // 2], f32, tag="sq")
                    zs = z_sb[:, h * HW + g * 784: h * HW + (g + 1) * 784]
                    qc = qsums[:, h * 4 + g:h * 4 + g + 1]
                    nc.scalar.activation(sq[:, 0:784], zs, AF.Square,
                                         accum_out=qc)
                elif g % 2 == 1:
                    ck = g // 2
                    sq = sqpool.tile([128, HW // 2], f32, tag="sq")
                    zs = z_sb[:, h * HW + ck * (HW // 2): h * HW + (ck + 1) * (HW // 2)]
                    nc.scalar.activation(sq[:], zs, AF.Square,
                                         accum_out=qsums[:, h * 4 + ck:h * 4 + ck + 1])

                if gi == 0 and prev is not None:
                    emit_stats_a(prev)
                elif gi == 1 and prev is not None:
                    emit_stats_b(prev)
                if gi % 2 == 1 and pending:
                    emit_norm(pending.pop(0))

        # tail: stats for the last sample, then drain all remaining chunks
        emit_stats_a(state["st"])
        emit_stats_b(state["st"])
        while pending:
            emit_norm(pending.pop(0))


def _build_program():
    import concourse.bacc as bacc
    import concourse.tile as tile
    import concourse.mybir as mybir

    f32 = mybir.dt.float32
    bf16 = mybir.dt.bfloat16

    nc = bacc.Bacc("TRN2", target_bir_lowering=False, debug=False, num_devices=1)

    v_d = nc.dram_tensor("v", [CIN, B * VS], bf16, kind="ExternalInput").ap()
    w_d = nc.dram_tensor("w", [CIN, 24 * 128], bf16, kind="ExternalInput").ap()
    bq_d = nc.dram_tensor("bq", [128, 3], f32, kind="ExternalInput").ap()
    y_d = nc.dram_tensor("y", [B, COUT, HW], bf16, kind="ExternalOutput").ap()

    with tile.TileContext(nc) as tc:
        _emit(nc, tc, v_d, w_d, bq_d, y_d)

    nc.compile()
    return nc


def _get_program():
    if "nc" not in _CACHE:
        _CACHE["nc"] = _build_program()
    return _CACHE["nc"]


def _binarize(t_np):
    """(t > t.mean()) as f32, matching the reference's jnp computation."""
    try:
        import jax.numpy as jnp
        tj = jnp.asarray(t_np)
        return np.asarray((tj > tj.mean()).astype(jnp.float32))
    except Exception:
        return (t_np > np.float32(t_np.astype(np.float64).mean())).astype(np.float32)


def kernel(x, weight, bias, train_mode=None):
    """Full-input entry point: shards over 8 NeuronCores, returns full output."""
    import time
    last_err = None
    for attempt in range(3):
        try:
            return _kernel_impl(x, weight, bias)
        except Exception as e:  # transient NRT/device errors: back off and retry
            last_err = e
            if attempt < 2:
                time.sleep(20.0 * (attempt + 1))
    raise last_err


def _kernel_impl(x, weight, bias):
    global LAST_RESULTS
    from concourse.bass_utils import run_bass_kernel_spmd

    x = np.asarray(x, dtype=np.float32)
    weight = np.asarray(weight, dtype=np.float32)
    bias = np.asarray(bias, dtype=np.float32)

    wq = _binarize(weight)                       # [256,128,3,3] {0,1}
    bq = _binarize(bias)                         # [256] {0,1}

    # transformed weights GW[h,k,kh][ci,co]; block order per half:
    # P: GW0 kh0-2, GW1 kh0-2 | Q: GW2 kh0-2 | R: GW1 kh0-2, -GW3 kh0-2
    Gm = np.array([[1, 0, 0], [.5, .5, .5], [.5, -.5, .5], [0, 0, 1]], np.float32)
    wqh = wq.reshape(2, 128, CIN, 3, 3)          # [h, co, ci, kh, kw]
    GW = np.einsum('kw,hoiyw->hkyio', Gm, wqh)   # [2, 4k, 3kh, ci, co]
    blocks = []
    for hh in range(2):
        for k, s in ((0, 1), (1, 1), (2, 1), (3, -1)):
            for kh in range(3):
                blocks.append(s * GW[hh, k, kh])         # [ci, co]
    wflat = np.ascontiguousarray(
        np.stack(blocks, axis=1).reshape(CIN, 24 * 128)
    ).astype(ml_dtypes.bfloat16)

    bq2 = np.zeros((128, 3), np.float32)
    bq2[:, 0] = bq[0:128]
    bq2[:, 1] = bq[128:256]
    bq2[:, 2] = HW * bq.sum()                    # C1 constant, replicated

    # input transform V = B^T d along width (host side, f32 then one
    # bf16 rounding): V[b,ci,y,k,t], k in {d0-d2, d1+d2, d2-d1, d1-d3}
    xp = np.zeros((B_FULL, CIN, HP, HP), np.float32)
    xp[:, :, 1:H + 1, 1:W + 1] = x
    d0 = xp[:, :, :, 0:56:2]
    d1 = xp[:, :, :, 1:57:2]
    d2 = xp[:, :, :, 2:58:2]
    d3 = xp[:, :, :, 3:58:2]
    V = np.empty((B_FULL, CIN, HP, 4, NT), np.float32)
    np.subtract(d0, d2, out=V[:, :, :, 0, :])
    np.add(d1, d2, out=V[:, :, :, 1, :])
    np.subtract(d2, d1, out=V[:, :, :, 2, :])
    np.subtract(d1, d3, out=V[:, :, :, 3, :])
    Vb = V.astype(ml_dtypes.bfloat16).reshape(B_FULL, CIN, VS)

    in_maps = []
    for c in range(N_CORES):
        vc = np.ascontiguousarray(
            Vb[c * B:(c + 1) * B].transpose(1, 0, 2).reshape(CIN, B * VS))
        in_maps.append({"v": vc, "w": wflat, "bq": bq2})

    nc = _get_program()
    kwargs = {}
    if TRACE:
        kwargs = dict(trace=True, tmpdir=TRACE_DIR)
    res = run_bass_kernel_spmd(nc, in_maps, core_ids=list(range(N_CORES)), **kwargs)
    LAST_RESULTS = res

    out = np.concatenate([np.asarray(res.results[c]["y"]).astype(np.float32)
                          for c in range(N_CORES)], axis=0)
    return out.reshape(B_FULL, COUT, H, W)
